# revision 1
# baseline (speedup 1.0000x reference)
"""Multi-head causal attention (B=2, S=2048, D=1024, H=16, Dh=64) on 8 TRN2
NeuronCores.

Sharding: tensor-parallel over heads — core c owns heads (2c, 2c+1) for both
batches. Each core projects Q^T/K^T/V for its 2 heads (inputs pre-transposed
on host so every DMA is a contiguous 2D transfer), runs causal attention in
transposed form (scores S^T = K Q^T per 128-k-tile, exp on ACT straight from
PSUM, diagonal-block causal masking, P@V as V_aug^T-stationary matmul whose
extra ones-column yields the softmax denominator for free), normalizes, then
all-to-alls attn^T chunks so core c holds all 1024 head-dims for query rows
[512c, 512c+512) and finishes the w_o projection locally. Host concatenates
the 8 row-slices.

All matmuls run as float32r (full PE rate at moving-dim >= 256, ~tf32
accuracy; validated ~2e-4 rel err on the attention micro-pipeline).
"""
import ml_dtypes
import numpy as np

import concourse.bass as bass
import concourse.mybir as mybir
import concourse.tile as tile
from concourse.bass_utils import run_bass_kernel_spmd

F32 = mybir.dt.float32
F32R = mybir.dt.float32r
BF16 = mybir.dt.bfloat16

B = 2
S = 2048
D = 1024
H = 16
DH = 64
N_CORES = 8
R = B * S          # 4096 global rows
RC = R // N_CORES  # 512 rows per core for the output projection

# ---------------------------------------------------------------------------
# BIR splitter: this toolchain's walrus rejects >1 sem-wait per instruction;
# move extra waits onto preceding same-engine nops (identical semantics).
def _split_waits(nc, maxw=1):
    for f in nc.m.functions:
        for bb in f.blocks:
            new_insts = []
            for ins in bb.instructions:
                si = ins.sync_info
                waits = list(si.on_wait) if si and si.on_wait else []
                if len(waits) > maxw:
                    carry, keep = waits[:-maxw], waits[-maxw:]
                    for j in range(0, len(carry), maxw):
                        new_insts.append(
                            mybir.InstNoOp(
                                name=f"{ins.name}-ws{j}",
                                engine=ins.engine,
                                sync_info=mybir.SyncInfo(
                                    on_wait=carry[j : j + maxw], on_update=[]
                                ),
                                bass_nofuse=True,
                            )
                        )
                    ins.sync_info = mybir.SyncInfo(
                        on_wait=keep,
                        on_update=list(si.on_update) if si.on_update else [],
                    )
                new_insts.append(ins)
            bb.instructions = new_insts


def _build():
    nc = bass.Bass()

    xT_d = nc.declare_dram_parameter("xT", [D, R], BF16, isOutput=False)
    wT_d = nc.declare_dram_parameter("wT", [D, 6 * DH], BF16, isOutput=False)
    woT_d = nc.declare_dram_parameter("woT", [D, D], BF16, isOutput=False)
    masks_d = nc.declare_dram_parameter("masks", [512, 512], BF16, isOutput=False)
    ident_d = nc.declare_dram_parameter("ident", [128, 128], BF16, isOutput=False)
    ones_d = nc.declare_dram_parameter("ones", [128, 64], BF16, isOutput=False)
    out_d = nc.declare_dram_parameter("out", [RC, D], F32, isOutput=True)

    a2a_in = [
        nc.dram_tensor(f"a2a_in{h}", [N_CORES, 64, RC], BF16) for h in range(2)
    ]
    a2a_out = [
        nc.dram_tensor(f"a2a_out{h}", [N_CORES, 64, RC], BF16) for h in range(2)
    ]
    denom_d = nc.dram_tensor("denom", [16, 512], F32)
    recip_d = nc.dram_tensor("recip", [16, 512], F32)

    NT = R // 512       # 8 column chunks of the projection
    NC_T = D // 128     # 8 contraction tiles
    NST = R // 128      # 32 s-tiles for V

    with tile.TileContext(nc) as tc:
      with nc.allow_low_precision(reason="bf16 attention pipeline"):
        with (
            tc.tile_pool(name="consts", bufs=1) as consts,
            tc.tile_pool(name="qk", bufs=1) as qk_pool,
            tc.tile_pool(name="vaug", bufs=1) as vaug_pool,
            tc.tile_pool(name="attn", bufs=1) as attn_pool,
            tc.tile_pool(name="work", bufs=6) as work,
            tc.tile_pool(name="norm", bufs=3) as norm_pool,
            tc.tile_pool(name="outp", bufs=2) as out_pool,
        ):
            masks = []
            ones_row = consts.tile([65, 64], F32R, tag="ones_row")
            nc.gpsimd.dma_start(out=ones_row, in_=ones_d[0:65, :])
            qT = qk_pool.tile([128, R], BF16, tag="qT")
            kT = qk_pool.tile([128, R], BF16, tag="kT")
            v_augs = [
                vaug_pool.tile([128, 130], BF16, tag=f"va{st}", name=f"va{st}")
                for st in range(NST)
            ]
            attnT = [
                attn_pool.tile([64, R], BF16, tag=f"attnT{h}", name=f"attnT{h}")
                for h in range(2)
            ]

            # ---- stage A: QKV projection (qT, kT, vT) ----------------------
            with (
                tc.tile_pool(name="wt", bufs=1) as wt_pool,
                tc.tile_pool(name="xs", bufs=3) as x_pool,
                tc.tile_pool(name="vt", bufs=1) as vt_pool,
                tc.tile_pool(name="psum_qkv", bufs=2, space="PSUM") as psum_qkv,
            ):
                wts = []
                for ct in range(NC_T):
                    wt = wt_pool.tile([128, 6 * DH], BF16, tag=f"wt{ct}")
                    nc.sync.dma_start(
                        out=wt, in_=wT_d[128 * ct : 128 * (ct + 1), :]
                    )
                    wts.append(wt)
                vT = vt_pool.tile([128, R], BF16, tag="vT")

                for n in range(NT):
                    xts = []
                    for ct in range(NC_T):
                        xt = x_pool.tile([128, 512], BF16, tag=f"x{ct}")
                        nc.sync.dma_start(
                            out=xt,
                            in_=xT_d[128 * ct : 128 * (ct + 1), 512 * n : 512 * (n + 1)],
                        )
                        xts.append(xt)
                    if n == 1:
                        # consts deferred out of the critical first DMA wave
                        ident = consts.tile([128, 128], BF16, tag="ident")
                        nc.sync.dma_start(out=ident, in_=ident_d[:, :])
                        for m in range(4):
                            mk = consts.tile(
                                [128, 512], BF16, tag=f"mask{m}", name=f"mask{m}"
                            )
                            nc.sync.dma_start(
                                out=mk, in_=masks_d[128 * m : 128 * (m + 1), :]
                            )
                            masks.append(mk)
                    for mi, dst in ((0, qT), (1, kT), (2, vT)):
                        ps = psum_qkv.tile([128, 512], F32, tag="qkvp")
                        for ct in range(NC_T):
                            nc.tensor.matmul(
                                ps,
                                lhsT=wts[ct][:, 128 * mi : 128 * (mi + 1)],
                                rhs=xts[ct],
                                start=(ct == 0),
                                stop=(ct == NC_T - 1),
                            )
                        nc.scalar.copy(dst[:, 512 * n : 512 * (n + 1)], ps)

                # ---- stage A2: V natural + ones columns --------------------
                for st in range(NST):
                    pt = psum_qkv.tile([128, 128], BF16, tag="vtp")
                    nc.tensor.transpose(
                        pt, vT[:, 128 * st : 128 * (st + 1)], ident
                    )
                    va = v_augs[st]
                    nc.vector.memset(va[:, 64:65], 1.0)
                    nc.vector.memset(va[:, 129:130], 1.0)
                    nc.scalar.copy(va[:, 0:64], pt[:, 0:64])
                    nc.scalar.copy(va[:, 65:129], pt[:, 64:128])

            with (
                tc.tile_pool(name="wo", bufs=1) as wo_pool,
                tc.tile_pool(name="af", bufs=1) as af_pool,
            ):
              # ---- w_o loads (overlap attention) ---------------------------
              wos = []
              for dt in range(NC_T):
                  wo = wo_pool.tile([128, D], BF16, tag=f"wo{dt}")
                  nc.sync.dma_start(out=wo, in_=woT_d[128 * dt : 128 * (dt + 1), :])
                  wos.append(wo)

              afs = {}   # (h, t) -> [128, RC] tile

              # ---- stage B: attention, h-outer so h0's a2a overlaps h1 -----
              with (
                  tc.tile_pool(name="psum_pv", bufs=1, space="PSUM") as psum_pv,
                  tc.tile_pool(name="psum_s", bufs=3, space="PSUM") as psum_s,
                  tc.tile_pool(name="psum_rb", bufs=1, space="PSUM") as psum_rb,
              ):
               for h in range(2):
                  hb = 64 * h
                  for b in range(B):
                      for qc in range(4):
                          q0 = 2048 * b + 512 * qc
                          nkt = 4 * qc + 4
                          pv = psum_pv.tile([65, 512], F32, tag="pv")
                          ng = nkt // 2
                          es = {}

                          def emit_s(g, b=b, h=h, qc=qc, hb=hb, q0=q0, es=es):
                              sp = psum_s.tile([128, 1024], F32, tag="sp")
                              for half in range(2):
                                  kt = 2 * g + half
                                  k0 = 2048 * b + 128 * kt
                                  nc.tensor.matmul(
                                      sp[:, 512 * half : 512 * (half + 1)],
                                      lhsT=kT[hb : hb + 64, k0 : k0 + 128],
                                      rhs=qT[hb : hb + 64, q0 : q0 + 512],
                                      start=True,
                                      stop=True,
                                  )
                              e2 = work.tile(
                                  [128, 1024], BF16, tag="expS", name=f"e{g}"
                              )
                              nc.scalar.activation(
                                  e2, sp, mybir.ActivationFunctionType.Exp,
                                  scale=0.125,
                              )
                              for half in range(2):
                                  kt = 2 * g + half
                                  m = kt - 4 * qc
                                  if m >= 0:
                                      nc.vector.tensor_mul(
                                          e2[:, 512 * half : 512 * (half + 1)],
                                          e2[:, 512 * half : 512 * (half + 1)],
                                          masks[m],
                                      )
                              es[g] = e2

                          for g0 in range(min(4, ng)):
                              emit_s(g0)
                          for g in range(0, ng, 2):
                              for gg in (g + 4, g + 5):
                                  if gg < ng:
                                      emit_s(gg)
                              # PV runs of 4 matmuls: longer same-kind bursts
                              # let the PE overlap drain with the next fill
                              for gsub in (g, g + 1):
                                  e2 = es.pop(gsub)
                                  for half in range(2):
                                      kt = 2 * gsub + half
                                      nc.tensor.matmul(
                                          pv,
                                          lhsT=v_augs[16 * b + kt][:, 65 * h : 65 * h + 65],
                                          rhs=e2[:, 512 * half : 512 * (half + 1)],
                                          start=(kt == 0),
                                          stop=(kt == nkt - 1),
                                      )
                          # evacuate + normalize (release pv bank fast)
                          pvc = norm_pool.tile([65, 512], F32, tag="pvc")
                          nc.vector.tensor_copy(pvc, pv)
                          lnd = norm_pool.tile([65, 512], F32, tag="lnd")
                          nc.scalar.activation(
                              lnd[64:65, :], pvc[64:65, :],
                              mybir.ActivationFunctionType.Ln,
                          )
                          rbuf = norm_pool.tile([65, 512], F32R, tag="rbuf")
                          nc.scalar.activation(
                              rbuf[64:65, :], lnd[64:65, :],
                              mybir.ActivationFunctionType.Exp, scale=-1.0,
                          )
                          rB = psum_rb.tile([64, 512], F32, tag="rB")
                          nc.tensor.matmul(
                              rB,
                              lhsT=ones_row[64:65, :],
                              rhs=rbuf[64:65, :],
                              start=True,
                              stop=True,
                          )
                          rB_sb = norm_pool.tile([64, 512], F32, tag="rB_sb")
                          nc.vector.tensor_copy(rB_sb, rB)
                          nc.gpsimd.tensor_mul(
                              attnT[h][:, q0 : q0 + 512], pvc[0:64, :], rB_sb
                          )

                  # ---- a2a for this head (h0's overlaps h1 compute) --------
                  for j in range(N_CORES):
                      nc.sync.dma_start(
                          out=a2a_in[h][j],
                          in_=attnT[h][:, 512 * j : 512 * (j + 1)],
                      )
                  nc.gpsimd.collective_compute(
                      "AllToAll",
                      mybir.AluOpType.bypass,
                      ins=[a2a_in[h][:]],
                      outs=[a2a_out[h][:]],
                      replica_groups=[list(range(N_CORES))],
                  )
                  flat = a2a_out[h][:].rearrange("a b c -> (a b) c")
                  for t in range(4):
                      af = af_pool.tile(
                          [128, RC], BF16, tag=f"af{h}_{t}", name=f"af{h}_{t}"
                      )
                      nc.sync.dma_start(
                          out=af, in_=flat[128 * t : 128 * (t + 1), :]
                      )
                      afs[(h, t)] = af

              # ---- output projection: emitted after both collectives are
              # issued; the h0-half matmuls run on PE while cc2 is in flight
              with tc.tile_pool(name="psum_o", bufs=2, space="PSUM") as psum_o:
               for stile in range(RC // 128):
                  ot = out_pool.tile([128, D], F32, tag="out")
                  for dc in range(2):
                      po = psum_o.tile([128, 512], F32, tag="po")
                      for hh in range(2):
                          for t in range(4):
                              nc.tensor.matmul(
                                  po,
                                  lhsT=afs[(hh, t)][:, 128 * stile : 128 * (stile + 1)],
                                  rhs=wos[4 * hh + t][:, 512 * dc : 512 * (dc + 1)],
                                  start=(hh == 0 and t == 0),
                                  stop=(hh == 1 and t == 3),
                              )
                      nc.vector.tensor_copy(ot[:, 512 * dc : 512 * (dc + 1)], po)
                  nc.sync.dma_start(
                      out=out_d[128 * stile : 128 * (stile + 1), :], in_=ot
                  )

    _split_waits(nc, maxw=1)
    return nc


def _install_ntff_shim():
    """Register the NTFF profile hook that this image's `antenv` lacks.

    bass_utils reads `antenv.axon_hooks.get_axon_ntff_profile_hook()` when
    trace=True under axon; provide the module via sys.modules and wire the
    ctypes hook against the axon PJRT .so (same ABI trn_boot uses).
    """
    import sys
    import types
    import ctypes
    import contextlib

    if "antenv.axon_hooks" in sys.modules:
        return
    so_path = "/opt/axon/libaxon_pjrt.so"
    try:
        lib = ctypes.CDLL(so_path)
    except OSError:
        return
    if not hasattr(lib, "axon_start_nrt_profile"):
        return
    lib.axon_start_nrt_profile.argtypes = [
        ctypes.POINTER(ctypes.c_int64),
        ctypes.c_size_t,
    ]
    lib.axon_start_nrt_profile.restype = ctypes.c_int64
    lib.axon_stop_nrt_profile.argtypes = [ctypes.c_char_p]
    lib.axon_stop_nrt_profile.restype = ctypes.c_int64

    @contextlib.contextmanager
    def _hook(output_dir, device_ids):
        import jax

        jax.devices()
        if device_ids:
            ids = (ctypes.c_int64 * len(device_ids))(*device_ids)
            rc = lib.axon_start_nrt_profile(ids, len(device_ids))
        else:
            rc = lib.axon_start_nrt_profile(None, 0)
        if rc != 0:
            raise RuntimeError(f"axon_start_nrt_profile rc={rc}")
        try:
            yield
        finally:
            n = lib.axon_stop_nrt_profile(str(output_dir).encode())
            print(f"ntff profile: {n} file(s) written to {output_dir}")

    mod = types.ModuleType("antenv.axon_hooks")
    mod.get_axon_ntff_profile_hook = lambda: _hook
    mod.set_axon_ntff_profile_hook = lambda h: None
    sys.modules["antenv.axon_hooks"] = mod


_nc_cache = None


def _get_nc():
    global _nc_cache
    if _nc_cache is None:
        _nc_cache = _build()
    return _nc_cache


def _prep_inputs(x, w_qkv, w_o):
    x = np.asarray(x, dtype=np.float32)
    w_qkv = np.asarray(w_qkv, dtype=np.float32)
    w_o = np.asarray(w_o, dtype=np.float32)

    bf = ml_dtypes.bfloat16
    xT = np.ascontiguousarray(x.reshape(R, D).T.astype(bf))   # [D, R]
    woT_full = w_o.T  # [d, d'] contraction rows
    # head-half reorder: rows with (d mod 128) < 64 (h0 of each core), then >= 64
    dd = np.arange(D)
    order = np.concatenate([dd[(dd % 128) < 64], dd[(dd % 128) >= 64]])
    woT = np.ascontiguousarray(woT_full[order].astype(bf))    # [D, D]

    w_q = w_qkv[0:D]
    w_k = w_qkv[D : 2 * D]
    w_v = w_qkv[2 * D : 3 * D]

    masks = np.zeros((4, 128, 512), ml_dtypes.bfloat16)
    kk = np.arange(128)[:, None]
    qq = np.arange(512)[None, :]
    for m in range(4):
        masks[m] = (qq >= kk + 128 * m).astype(ml_dtypes.bfloat16)
    masks = masks.reshape(512, 512)

    ident = np.eye(128, dtype=ml_dtypes.bfloat16)
    ones = np.ones((128, 64), ml_dtypes.bfloat16)

    in_maps = []
    for c in range(N_CORES):
        h0, h1 = 2 * c, 2 * c + 1
        cols = []
        for w in (w_q, w_k, w_v):
            cols.append(w[DH * h0 : DH * h0 + DH])
            cols.append(w[DH * h1 : DH * h1 + DH])
        # [6*DH, D] rows: q_h0,q_h1,k_h0,k_h1,v_h0,v_h1 -> transpose to [D, 6*DH]
        w_slice = np.concatenate(cols, axis=0)
        wT = np.ascontiguousarray(w_slice.T.astype(ml_dtypes.bfloat16))
        in_maps.append(
            {
                "xT": xT,
                "wT": wT,
                "woT": woT,
                "masks": masks,
                "ident": ident,
                "ones": ones,
            }
        )
    return in_maps


def kernel(x, w_qkv, w_o, _trace=False):
    if _trace:
        _install_ntff_shim()
    nc = _get_nc()
    in_maps = _prep_inputs(x, w_qkv, w_o)
    res = run_bass_kernel_spmd(
        nc, in_maps, list(range(N_CORES)), trace=_trace
    )
    out = np.concatenate(
        [res.results[c]["out"] for c in range(N_CORES)], axis=0
    )  # [R, D]
    out = out.reshape(B, S, D)
    if _trace:
        kernel.last_exec_time_ns = res.exec_time_ns
        kernel.last_results = res
    return out

